# revision 1
# baseline (speedup 1.0000x reference)
"""Trainium2 Bass kernel for MiniBatch Edge-Conditioned Conv (2 blocks + classifier).

Reference computation (see problem):
  block(h, ef, We, be, Wn, bn, src, dst, nid, n_dst):
    e   = relu(ef @ We + be).reshape(E, H, D)      # per-edge weights
    m   = einsum('ehd,ed->eh', e, h[src])          # per-edge matvec
    agg = segment_sum(m, dst, n_dst)
    return agg + relu(h[nid] @ Wn + bn)
  out = block1(block0(nf)) @ Wfc + bfc

Sharding: edges sorted by dst, sharded by dst-range across 8 cores (so the
segment-sum is core-local).  h1 is AllGathered between blocks.

Device pipeline per 128-edge chunk:
  PE:  P = efT_aug.T @ We_aug            [128e, (h,d)] in PSUM (K=17, bias folded)
  ACT/DVE: Pr = relu(P) -> SBUF bf16     (PSUM evacuation)
  DVE: T = Pr * hs[e,d] (bcast over h)   bf16
  PE:  seg_psum[v, (h,d)] += onehot[e,v].T @ T    (accumulated over chunks)
  DVE: agg[v, h] = reduce_d(seg_psum)
"""

import math
import sys

sys.path.insert(0, "/opt/trn_rl_repo")

import numpy as np
import ml_dtypes

import concourse.bass as bass
import concourse.mybir as mybir
import concourse.tile as tile
from concourse import bacc, bass_utils

BF16 = ml_dtypes.bfloat16

# Problem constants (hardcoded per harness contract)
N0, N1, N2 = 102400, 10240, 1024
D_IN, E_IN, H, C = 64, 16, 64, 10
E0, E1 = 102400, 10240
NCORES = 8
P = 128
HD = H * D_IN  # 4096

PAD_SENTINEL = 200.0
DIAG_NO_GATHER = False
DIAG_NO_CC = False
DIAG_NO_MULT = False
DIAG_NO_SEG = False  # dst-local value for padding edges -> zero one-hot row


def _prep_edges(ef, src, dst, n_dst_per_core, tiles_per_core):
    """Sort edges by dst, shard by dst-range, pad per (core,tile) to chunks of 128.

    Returns per-core lists of arrays + per-tile chunk counts (shared by cores).
    """
    E = ef.shape[0]
    core = dst // n_dst_per_core
    tloc = (dst % n_dst_per_core) // P
    dloc = dst % P

    counts = np.zeros((NCORES, tiles_per_core), dtype=np.int64)
    np.add.at(counts, (core, tloc), 1)
    cpts = np.maximum(1, np.ceil(counts.max(axis=0) / P).astype(np.int64))  # [T]
    offs = np.concatenate([[0], np.cumsum(cpts)])  # chunk offsets per tile
    total_chunks = int(offs[-1])
    EP = total_chunks * P

    # bucket edge ids per (core, tile)
    order = np.lexsort((dloc, tloc, core))
    sc, st = core[order], tloc[order]
    # boundaries
    eftA = np.zeros((NCORES, 17, EP), dtype=BF16)
    srcA = np.zeros((NCORES, P, total_chunks), dtype=np.int32)
    dstA = np.full((NCORES, P, total_chunks), PAD_SENTINEL, dtype=np.float32)

    ef16 = ef.astype(BF16)
    idx_all = np.arange(E)
    for c in range(NCORES):
        for t in range(tiles_per_core):
            sel = order[(sc == c) & (st == t)]
            n = len(sel)
            width = int(cpts[t]) * P
            col0 = int(offs[t]) * P
            # positions inside this tile's padded block, chunk-major:
            # edge j -> chunk j//P? We want column layout [P, cpts]: edge j ->
            # (p=j%P, ch=j//P). eft columns are (ch*P + p).
            eftA[c, :16, col0 : col0 + n] = ef16[sel].T
            eftA[c, 16, col0 : col0 + n] = 1.0
            ch = idx_all[:n] // P
            pp = idx_all[:n] % P
            srcA[c, pp, int(offs[t]) + ch] = src[sel]
            dstA[c, pp, int(offs[t]) + ch] = (dst[sel] % P).astype(np.float32)
    return eftA, srcA, dstA, cpts, offs, EP, total_chunks


def _augment(W, b):
    return np.concatenate([W, b[None, :]], axis=0).astype(BF16)


def _build_program(cpts0, offs0, EP0, TC0, cpts1, offs1, EP1, TC1):
    """Build the SPMD Bass program (same NEFF for all 8 cores)."""
    nc = bacc.Bacc(
        "TRN2", target_bir_lowering=False, debug=False,
        num_devices=1 if DIAG_NO_CC else NCORES,
    )
    dt = mybir.dt
    T0 = N1 // NCORES // P  # 10 dst tiles per core, block 0

    # ---- I/O ----
    i_we0 = nc.dram_tensor("we0a", [17, HD], dt.bfloat16, kind="ExternalInput")
    i_we1 = nc.dram_tensor("we1a", [17, HD], dt.bfloat16, kind="ExternalInput")
    i_wn0 = nc.dram_tensor("wn0a", [D_IN + 1, H], dt.bfloat16, kind="ExternalInput")
    i_wn1 = nc.dram_tensor("wn1a", [H + 1, H], dt.bfloat16, kind="ExternalInput")
    i_wfc = nc.dram_tensor("wfca", [H + 1, C], dt.bfloat16, kind="ExternalInput")
    i_nf = nc.dram_tensor("nf16", [N0, D_IN], dt.bfloat16, kind="ExternalInput")
    i_eft0 = nc.dram_tensor("eft0", [17, EP0], dt.bfloat16, kind="ExternalInput")
    i_src0 = nc.dram_tensor("src0i", [P, TC0], dt.int32, kind="ExternalInput")
    i_dst0 = nc.dram_tensor("dstl0", [P, TC0], dt.float32, kind="ExternalInput")
    i_eft1 = nc.dram_tensor("eft1", [17, EP1], dt.bfloat16, kind="ExternalInput")
    i_src1 = nc.dram_tensor("src1i", [P, TC1], dt.int32, kind="ExternalInput")
    i_dst1 = nc.dram_tensor("dstl1", [P, TC1], dt.float32, kind="ExternalInput")
    i_nid0 = nc.dram_tensor("nidx0", [P, T0], dt.int32, kind="ExternalInput")
    i_nid1 = nc.dram_tensor("nidx1", [P, 1], dt.int32, kind="ExternalInput")
    i_iota = nc.dram_tensor("iota", [P, P], dt.bfloat16, kind="ExternalInput")
    i_ident = nc.dram_tensor("ident", [P, P], dt.bfloat16, kind="ExternalInput")
    o_out = nc.dram_tensor("out", [P, C], dt.float32, kind="ExternalOutput")

    RELU = mybir.ActivationFunctionType.Relu
    MULT = mybir.AluOpType.mult
    ISEQ = mybir.AluOpType.is_equal
    ADD = mybir.AluOpType.add

    with tile.TileContext(nc) as tc:
        with (
            tc.tile_pool(name="const", bufs=1) as cpool,
            tc.tile_pool(name="dram", bufs=1, space="DRAM") as dpool,
            tc.tile_pool(name="agg", bufs=1) as apool,
        ):
            we0_s = cpool.tile([17, HD], dt.bfloat16)
            nc.sync.dma_start(we0_s[:], i_we0[:])
            we1_s = cpool.tile([17, HD], dt.bfloat16)
            nc.sync.dma_start(we1_s[:], i_we1[:])
            wn0_s = cpool.tile([D_IN + 1, H], dt.bfloat16)
            nc.sync.dma_start(wn0_s[:], i_wn0[:])
            wn1_s = cpool.tile([H + 1, H], dt.bfloat16)
            nc.sync.dma_start(wn1_s[:], i_wn1[:])
            wfc_s = cpool.tile([H + 1, C], dt.bfloat16)
            nc.sync.dma_start(wfc_s[:], i_wfc[:])
            iota_s = cpool.tile([P, P], dt.bfloat16)
            nc.sync.dma_start(iota_s[:], i_iota[:])
            ident_s = cpool.tile([P, P], dt.bfloat16)
            nc.sync.dma_start(ident_s[:], i_ident[:])

            h1s = dpool.tile([N1 // NCORES, H], dt.bfloat16)  # own slice
            h1f = dpool.tile([N1, H], dt.bfloat16)  # all-gathered

            agg0 = apool.tile([P, T0 * H], dt.float32)
            agg1 = apool.tile([P, H], dt.float32)

            def edge_phase(Ttiles, cpts, offs, eft_in, src_in, dst_in, we_s,
                           gather_dram, agg_tile):
                """Edge pipeline; writes agg_tile[:, t*H:(t+1)*H] per dst tile."""
                max_cpt = max(int(cpts[t]) for t in range(Ttiles))
                with (
                    tc.tile_pool(name="chunkin", bufs=3) as chpool,
                    tc.tile_pool(name="hsp", bufs=max_cpt + 2) as hspool,
                    tc.tile_pool(name="ohp", bufs=max_cpt + 2) as ohpool,
                    tc.tile_pool(name="work", bufs=8) as wpool,
                    tc.tile_pool(name="genps", bufs=2, space="PSUM") as gpool,
                    tc.tile_pool(name="segps", bufs=2, space="PSUM") as segpool,
                ):
                    for t in range(Ttiles):
                        cpt = int(cpts[t])
                        ch0 = int(offs[t])
                        # per-tile caches
                        eft_c = chpool.tile([17, cpt * P], dt.bfloat16, tag="eft")
                        nc.sync.dma_start(
                            eft_c[:], eft_in[:, ch0 * P : (ch0 + cpt) * P]
                        )
                        src_c = chpool.tile([P, cpt], dt.int32, tag="src")
                        nc.sync.dma_start(src_c[:], src_in[:, ch0 : ch0 + cpt])
                        dst_c = chpool.tile([P, cpt], dt.float32, tag="dst")
                        nc.sync.dma_start(dst_c[:], dst_in[:, ch0 : ch0 + cpt])

                        hs_list = []
                        oh_list = []
                        for ch in range(cpt):
                            hs_ch = hspool.tile([P, D_IN], dt.bfloat16, tag="hs")
                            if DIAG_NO_GATHER:
                                nc.vector.memset(hs_ch[:], 1.0)
                            else:
                                nc.gpsimd.indirect_dma_start(
                                    out=hs_ch[:],
                                    out_offset=None,
                                    in_=gather_dram[:],
                                    in_offset=bass.IndirectOffsetOnAxis(
                                        ap=src_c[:, ch : ch + 1], axis=0
                                    ),
                                )
                            oh_ch = ohpool.tile([P, P], dt.bfloat16, tag="oh")
                            nc.vector.tensor_scalar(
                                out=oh_ch[:],
                                in0=iota_s[:],
                                scalar1=dst_c[:, ch : ch + 1],
                                scalar2=None,
                                op0=ISEQ,
                            )
                            hs_list.append(hs_ch)
                            oh_list.append(oh_ch)

                        for unit in range(4):  # quarter passes of 1024 (h,d) cols
                            col0 = unit * 1024
                            seg = segpool.tile([P, 1024], dt.float32, tag="seg")
                            for ch in range(cpt):
                                g = gpool.tile([P, 1024], dt.float32, tag="g")
                                for q in range(2):
                                    nc.tensor.matmul(
                                        g[:, q * 512 : (q + 1) * 512],
                                        lhsT=eft_c[:, ch * P : (ch + 1) * P],
                                        rhs=we_s[:, col0 + q * 512 : col0 + (q + 1) * 512],
                                        start=True,
                                        stop=True,
                                    )
                                hs3 = (
                                    hs_list[ch][:]
                                    .rearrange("p (o d) -> p o d", o=1)
                                    .to_broadcast([P, 16, D_IN])
                                )
                                if unit == 1 and not DIAG_NO_MULT:
                                    # DVE: fused relu+mult straight from PSUM
                                    # (1 of 4 units; DVE also does the other
                                    # units' mults)
                                    tq = wpool.tile([P, 1024], dt.bfloat16, tag="tq")
                                    nc.vector.scalar_tensor_tensor(
                                        out=tq[:].rearrange("p (h d) -> p h d", d=D_IN),
                                        in0=g[:].rearrange("p (h d) -> p h d", d=D_IN),
                                        scalar=0.0,
                                        in1=hs3,
                                        op0=mybir.AluOpType.max,
                                        op1=MULT,
                                    )
                                else:
                                    pr = wpool.tile([P, 1024], dt.bfloat16, tag="pr")
                                    if unit == 1:
                                        nc.vector.tensor_scalar_max(pr[:], g[:], 0.0)
                                    else:
                                        # ACT evacuation (3 of 4 units)
                                        nc.scalar.activation(pr[:], g[:], RELU)
                                    if DIAG_NO_MULT:
                                        tq = pr
                                    else:
                                        tq = wpool.tile([P, 1024], dt.bfloat16, tag="tq")
                                        nc.vector.tensor_tensor(
                                            out=tq[:].rearrange("p (h d) -> p h d", d=D_IN),
                                            in0=pr[:].rearrange("p (h d) -> p h d", d=D_IN),
                                            in1=hs3,
                                            op=MULT,
                                        )
                                if not DIAG_NO_SEG:
                                    for q in range(2):
                                        nc.tensor.matmul(
                                            seg[:, q * 512 : (q + 1) * 512],
                                            lhsT=oh_list[ch][:],
                                            rhs=tq[:, q * 512 : (q + 1) * 512],
                                            start=(ch == 0),
                                            stop=(ch == cpt - 1),
                                        )
                                elif ch == 0:
                                    nc.tensor.matmul(
                                        seg[:, :512],
                                        lhsT=oh_list[ch][:],
                                        rhs=tq[:, :512],
                                        start=True,
                                        stop=True,
                                    )
                            # reduce over d: [128v, 16h, 64d] -> [128v, 16h]
                            nc.vector.tensor_reduce(
                                out=agg_tile[:, t * H + unit * 16 : t * H + unit * 16 + 16],
                                in_=seg[:].rearrange("p (h d) -> p h d", d=D_IN),
                                axis=mybir.AxisListType.X,
                                op=ADD,
                            )

            def node_update(Ttiles, nid_in, gather_dram, wn_s, agg_tile, out_cb):
                """h_out = agg + relu(gather[nid] @ Wn_aug); out_cb(t, tile[P,H] f32->bf16)."""
                with (
                    tc.tile_pool(name="nu", bufs=2) as npool,
                    tc.tile_pool(name="nups", bufs=2, space="PSUM") as npsum,
                ):
                    nid_c = npool.tile([P, Ttiles], dt.int32, tag="nid")
                    nc.sync.dma_start(nid_c[:], nid_in[:])
                    for t in range(Ttiles):
                        nfg = npool.tile([P, D_IN], dt.bfloat16, tag="nfg")
                        nc.gpsimd.indirect_dma_start(
                            out=nfg[:],
                            out_offset=None,
                            in_=gather_dram[:],
                            in_offset=bass.IndirectOffsetOnAxis(
                                ap=nid_c[:, t : t + 1], axis=0
                            ),
                        )
                        trp = npsum.tile([D_IN, P], dt.bfloat16, tag="trp")
                        nc.tensor.transpose(trp[:], nfg[:], ident_s[:])
                        nfgT = npool.tile([D_IN + 1, P], dt.bfloat16, tag="nfgT")
                        nc.vector.tensor_copy(nfgT[:D_IN, :], trp[:])
                        nc.vector.memset(nfgT[D_IN : D_IN + 1, :], 1.0)
                        nup = npsum.tile([P, H], dt.float32, tag="nup")
                        nc.tensor.matmul(
                            nup[:], lhsT=nfgT[:], rhs=wn_s[:], start=True, stop=True
                        )
                        nur = npool.tile([P, H], dt.float32, tag="nur")
                        nc.scalar.activation(nur[:], nup[:], RELU)
                        hout = npool.tile([P, H], dt.float32, tag="hout")
                        nc.vector.tensor_tensor(
                            out=hout[:],
                            in0=nur[:],
                            in1=agg_tile[:, t * H : (t + 1) * H],
                            op=ADD,
                        )
                        out_cb(t, hout, npool)

            # ================= BLOCK 0 =================
            T0n = N1 // NCORES // P
            edge_phase(T0n, cpts0, offs0, i_eft0, i_src0, i_dst0, we0_s, i_nf, agg0)

            def b0_out(t, hout, npool):
                hb = npool.tile([P, H], dt.bfloat16, tag="hb")
                nc.vector.tensor_copy(hb[:], hout[:])
                nc.sync.dma_start(h1s[t * P : (t + 1) * P, :], hb[:])

            node_update(T0n, i_nid0, i_nf, wn0_s, agg0, b0_out)

            # ================= ALLGATHER h1 =================
            if DIAG_NO_CC:
                nc.sync.dma_start(h1f[0 : N1 // NCORES, :], h1s[:])
                nc.sync.dma_start(h1f[N1 // NCORES :, :], 
                                  h1f[0 : N1 - N1 // NCORES, :])
            else:
                nc.gpsimd.collective_compute(
                    "AllGather",
                    mybir.AluOpType.bypass,
                    replica_groups=[list(range(NCORES))],
                    ins=[h1s[:].opt()],
                    outs=[h1f[:].opt()],
                )

            # ================= BLOCK 1 =================
            edge_phase(1, cpts1, offs1, i_eft1, i_src1, i_dst1, we1_s, h1f, agg1)

            def b1_out(t, hout, npool):
                # classifier: out = (h2_aug).T? -> transpose h2, matmul wfc
                hb = npool.tile([P, H], dt.bfloat16, tag="hb2")
                nc.vector.tensor_copy(hb[:], hout[:])
                with tc.tile_pool(name="fcps", bufs=1, space="PSUM") as fpool:
                    trp2 = fpool.tile([H, P], dt.bfloat16)
                    nc.tensor.transpose(trp2[:], hb[:], ident_s[:])
                    h2T = npool.tile([H + 1, P], dt.bfloat16, tag="h2T")
                    nc.vector.tensor_copy(h2T[:H, :], trp2[:])
                    nc.vector.memset(h2T[H : H + 1, :], 1.0)
                    ops = fpool.tile([P, C], dt.float32)
                    nc.tensor.matmul(
                        ops[:], lhsT=h2T[:], rhs=wfc_s[:], start=True, stop=True
                    )
                    osb = npool.tile([P, C], dt.float32, tag="osb")
                    nc.vector.tensor_copy(osb[:], ops[:])
                    nc.sync.dma_start(o_out[:], osb[:])

            node_update(1, i_nid1, h1f, wn1_s, agg1, b1_out)

    nc.compile()
    return nc


_CACHE = {}


def kernel(**inputs):
    node_features = np.asarray(inputs["node_features"], dtype=np.float32)
    ef0 = np.asarray(inputs["edge_feat0"], dtype=np.float32)
    ef1 = np.asarray(inputs["edge_feat1"], dtype=np.float32)
    We0 = np.asarray(inputs["We0"], dtype=np.float32)
    be0 = np.asarray(inputs["be0"], dtype=np.float32)
    We1 = np.asarray(inputs["We1"], dtype=np.float32)
    be1 = np.asarray(inputs["be1"], dtype=np.float32)
    Wn0 = np.asarray(inputs["Wn0"], dtype=np.float32)
    bn0 = np.asarray(inputs["bn0"], dtype=np.float32)
    Wn1 = np.asarray(inputs["Wn1"], dtype=np.float32)
    bn1 = np.asarray(inputs["bn1"], dtype=np.float32)
    Wfc = np.asarray(inputs["Wfc"], dtype=np.float32)
    bfc = np.asarray(inputs["bfc"], dtype=np.float32)
    src0 = np.asarray(inputs["src0"]).astype(np.int64)
    dst0 = np.asarray(inputs["dst0"]).astype(np.int64)
    src1 = np.asarray(inputs["src1"]).astype(np.int64)
    dst1 = np.asarray(inputs["dst1"]).astype(np.int64)
    nid0 = np.asarray(inputs["nid0"]).astype(np.int64)
    nid1 = np.asarray(inputs["nid1"]).astype(np.int64)

    T0 = N1 // NCORES // P  # 10
    eftA0, srcA0, dstA0, cpts0, offs0, EP0, TC0 = _prep_edges(ef0, src0, dst0, N1 // NCORES, T0)
    eftA1, srcA1, dstA1, cpts1, offs1, EP1, TC1 = _prep_edges(ef1, src1, dst1, N2 // NCORES, 1)

    key = (EP0, TC0, EP1, TC1, tuple(cpts0), tuple(cpts1))
    if key not in _CACHE:
        _CACHE[key] = _build_program(cpts0, offs0, EP0, TC0, cpts1, offs1, EP1, TC1)
    nc = _CACHE[key]

    we0a = _augment(We0, be0)
    we1a = _augment(We1, be1)
    wn0a = _augment(Wn0, bn0)
    wn1a = _augment(Wn1, bn1)
    wfca = _augment(Wfc, bfc)
    nf16 = node_features.astype(BF16)
    iota = np.broadcast_to(np.arange(P, dtype=np.float32), (P, P)).astype(BF16)
    ident = np.eye(P, dtype=np.float32).astype(BF16)

    in_maps = []
    for c in range(NCORES):
        nid0_c = nid0[c * (N1 // NCORES) : (c + 1) * (N1 // NCORES)]
        nid1_c = nid1[c * (N2 // NCORES) : (c + 1) * (N2 // NCORES)]
        in_maps.append(
            {
                "we0a": we0a,
                "we1a": we1a,
                "wn0a": wn0a,
                "wn1a": wn1a,
                "wfca": wfca,
                "nf16": nf16,
                "eft0": eftA0[c],
                "src0i": srcA0[c],
                "dstl0": dstA0[c],
                "eft1": eftA1[c],
                "src1i": srcA1[c],
                "dstl1": dstA1[c],
                "nidx0": nid0_c.reshape(T0, P).T.astype(np.int32).copy(),
                "nidx1": nid1_c.reshape(1, P).T.astype(np.int32).copy(),
                "iota": iota,
                "ident": ident,
            }
        )

    global last_results, _LAST_IN_MAPS
    _LAST_IN_MAPS = in_maps
    res = bass_utils.run_bass_kernel_spmd(nc, in_maps, core_ids=list(range(NCORES)))
    last_results = res
    out = np.concatenate([res.results[c]["out"] for c in range(NCORES)], axis=0)
    return out.astype(np.float32)


last_results = None


def bench(inputs, iters=8):
    """Time the compiled SPMD executable with device-resident inputs.

    Returns (best_seconds, list_of_seconds). Mirrors
    bass2jax.run_bass_via_pjrt's sharded-jit construction so the jitted fn
    is built once and timed with inputs already on device.
    """
    import time
    import jax
    from jax.sharding import Mesh, PartitionSpec, NamedSharding
    from jax.experimental.shard_map import shard_map
    from concourse import bass2jax, mybir as _mb

    # run once through kernel() to populate _CACHE and build in_maps
    kernel(**inputs)
    nc = next(iter(_CACHE.values()))
    in_maps = _LAST_IN_MAPS

    bass2jax.install_neuronx_cc_hook()
    partition_name = (
        nc.partition_id_tensor.name if nc.partition_id_tensor else None
    )
    in_names, out_names, out_avals, zero_outs = [], [], [], []
    for alloc in nc.m.functions[0].allocations:
        if not isinstance(alloc, _mb.MemoryLocationSet):
            continue
        name = alloc.memorylocations[0].name
        if alloc.kind == "ExternalInput":
            if name != partition_name:
                in_names.append(name)
        elif alloc.kind == "ExternalOutput":
            shape = tuple(alloc.tensor_shape)
            dtype = _mb.dt.np(alloc.dtype)
            out_avals.append(jax.core.ShapedArray(shape, dtype))
            out_names.append(name)
            zero_outs.append(np.zeros(shape, dtype))
    n_params = len(in_names)
    n_outs = len(out_avals)
    all_in_names = list(in_names) + list(out_names)
    if partition_name is not None:
        all_in_names.append(partition_name)
    donate = tuple(range(n_params, n_params + n_outs))

    def _body(*args):
        operands = list(args)
        if partition_name is not None:
            operands.append(bass2jax.partition_id_tensor())
        outs = bass2jax._bass_exec_p.bind(
            *operands,
            out_avals=tuple(out_avals),
            in_names=tuple(all_in_names),
            out_names=tuple(out_names),
            lowering_input_output_aliases=(),
            sim_require_finite=True,
            sim_require_nnan=True,
            nc=nc,
        )
        return tuple(outs)

    devices = jax.devices()[:NCORES]
    mesh = Mesh(np.asarray(devices), ("core",))
    in_specs = (PartitionSpec("core"),) * (n_params + n_outs)
    out_specs = (PartitionSpec("core"),) * n_outs
    sharded = jax.jit(
        shard_map(
            _body, mesh=mesh, in_specs=in_specs, out_specs=out_specs,
            check_rep=False,
        ),
        donate_argnums=donate,
        keep_unused=True,
    )
    shd = NamedSharding(mesh, PartitionSpec("core"))
    concat_in = [
        jax.device_put(
            np.concatenate([np.asarray(in_maps[c][n]) for c in range(NCORES)], axis=0),
            shd,
        )
        for n in in_names
    ]
    def zeros_dev():
        return [
            jax.device_put(
                np.zeros((NCORES * z.shape[0], *z.shape[1:]), z.dtype), shd
            )
            for z in zero_outs
        ]

    # warmup (compiles)
    o = sharded(*concat_in, *zeros_dev())
    jax.block_until_ready(o)
    times = []
    for _ in range(iters):
        zs = zeros_dev()
        jax.block_until_ready(zs)
        t0 = time.perf_counter()
        o = sharded(*concat_in, *zs)
        jax.block_until_ready(o)
        times.append(time.perf_counter() - t0)
    return min(times), times


if __name__ == "__main__":
    import reference

    inp = {k: np.asarray(v) for k, v in reference.setup_inputs().items()}
    expected = np.asarray(reference.reference(**reference.setup_inputs()))
    actual = kernel(**inp)
    err = np.abs(actual - expected).max() / (np.abs(expected).max() + 1e-9)
    print("Relative error:", err)

